# revision 25
# baseline (speedup 1.0000x reference)
"""Trainium2 Bass kernel for a 4-layer transformer encoder.

Model (hardcoded from the problem spec):
  L=4 layers, B=4, S=2048, D=512, H=8 heads (DH=64), FF=2048,
  inference BatchNorm with moving stats (0,1): bn(x) = x/sqrt(1+eps)*g + be.

Sharding: 8 cores. Cores (2b, 2b+1) handle batch item b; each computes
1024 of the item's 2048 tokens.  The residual stream is kept
feature-major [D, T] in SBUF (f32).  After each layer the pair
exchanges its updated activations (fp8) via a 2-rank AllGather through
DRAM bounce buffers so both cores have the full 2048-token sequence
for the next layer's K/V; the AllGather for (layer+1, qt) is issued as
soon as this layer's FFN for qt completes, hiding its latency under
the remaining attention / FFN work.

Precision strategy: the residual stream and all BN math stay f32; the
FFN and o-projection matmuls are bf16; Q/K/V projections, attention
scores and att@V run in fp8e4 with DoubleRow perf mode (two 128-deep
k-tiles per PE pass) for 2x matmul throughput.  fp8 errors inside
attention wash out in the softmax average and land on the (small)
sublayer outputs, not the residual stream.

Attention: scores are computed transposed [ktok, qtok] (contraction
DH=64; the two heads of a pair run concurrently on the top/bottom
halves of the PE array via row tiling), exp on ScalarE over [128,1024]
two-PSUM-bank tiles (scale folded in; logits are O(1) so no max
subtraction) emitting fp8, and att@V uses DoubleRow with a
ones-augmented, 128-column-per-head V so the softmax denominator
falls out of the same matmul: even heads occupy V columns 0..63 with
ones at column 64, odd heads have ones at column 63 and V at 64..127,
so the normalized outputs land on disjoint partition halves of attnT
and the o-projection contracts a full K=128 head pair per pass.
"""

import math

import numpy as np
import ml_dtypes

# ---- problem constants --------------------------------------------------
L, B, S, D, H = 4, 4, 2048, 512, 8
DH = D // H            # 64
FF = 4 * D             # 2048
BN_EPS = 1e-3
P = 128
DT = D // P            # 4 feature tiles
FT = FF // P           # 16 ffn tiles
HP = H // 2            # 4 head pairs
ATT_SCALE = 1.0 / math.sqrt(DH)
BN_INV = 1.0 / math.sqrt(1.0 + BN_EPS)
# fp8 weight pre-scale: raw qkv weights (std ~0.02) sit in e4m3's subnormal
# range; scale them up before casting, compensate via the exp() scale (q/k)
# and a folded 1/WS in wo (v path).
WS = 32.0
import os
EXP2BANK = os.environ.get("EXP2BANK", "1") == "1"

N_CORES = 8

BF16 = ml_dtypes.bfloat16
FP8 = ml_dtypes.float8_e4m3


def _fmajor(a, t):
    """[T, D_any] -> feature-major [128, D_any//128, T] tile layout."""
    d = a.shape[1]
    return np.ascontiguousarray(a.T.reshape(d // P, P, t).transpose(1, 0, 2))


def _w_tiles(w):
    """[K, N] weight -> [128, K//128, N] (partition = K within tile)."""
    k, n = w.shape
    return np.ascontiguousarray(w.reshape(k // P, P, n).transpose(1, 0, 2))


def _wo_tiles(w):
    """[D, D] attention out-proj -> [128, HP, D] with partition =
    dh + 64*(h%2), tile index = h//2 (matches attnT's layout)."""
    t = np.zeros((P, HP, D), w.dtype)
    for h in range(H):
        t[64 * (h % 2) : 64 * (h % 2) + 64, h // 2, :] = w[h * DH : (h + 1) * DH, :]
    return t


def _vec_tiles(v):
    """[L, D_any] -> [L, 128, D_any//128] per-partition layout."""
    l, d = v.shape
    return np.ascontiguousarray(v.reshape(l, d // P, P).transpose(0, 2, 1))


def build_encoder(nc, tc, *, n_layers, t_own, s_kv, use_ag):
    """Emit the encoder onto TileContext tc.  t_own = tokens this core
    computes; s_kv = tokens attended over (= 2*t_own when use_ag)."""
    import concourse.bass as bass
    import concourse.mybir as mybir
    from concourse.bass import ds, ts

    F32 = mybir.dt.float32
    BF = mybir.dt.bfloat16
    F8 = mybir.dt.float8e4
    AF = mybir.ActivationFunctionType
    OP = mybir.AluOpType
    DR = mybir.MatmulPerfMode.DoubleRow

    KT = s_kv // P          # 16 ktok 128-tiles
    KTP = KT // 2           # 8 ktok 256-pairs (DoubleRow step)
    SKT = s_kv // 512       # 4 kv projection stream tiles
    QT = t_own // 512       # 2 qtok stream tiles
    # process k-token stream chunks in exchange-arrival order: the qt0
    # AllGather is prefetched a layer earlier and lands first, the qt1 one
    # only finishes ~20us into the layer.
    ST_ORDER = [0, 2, 1, 3] if SKT == 4 else list(range(SKT))
    ARR = {st: i for i, st in enumerate(ST_ORDER)}
    KTP_ORDER = [k for st in ST_ORDER for k in (2 * st, 2 * st + 1)]
    # step index at which k-stream chunk st is first consumed
    ST_STEP = {st: 2 * i for i, st in enumerate(ST_ORDER)}

    # ---- dram I/O -------------------------------------------------------
    x0_d = nc.dram_tensor("x0", [P, DT, t_own], F32, kind="ExternalInput").ap()
    xkv0_d = nc.dram_tensor("xkv0", [P, DT, s_kv], F8, kind="ExternalInput").ap()
    wq_d = nc.dram_tensor("wq", [n_layers, P, DT, D], F8, kind="ExternalInput").ap()
    wk_d = nc.dram_tensor("wk", [n_layers, P, DT, D], F8, kind="ExternalInput").ap()
    wv_d = nc.dram_tensor("wv", [n_layers, P, DT, D], F8, kind="ExternalInput").ap()
    wo_d = nc.dram_tensor("wo", [n_layers, P, HP, D], BF, kind="ExternalInput").ap()
    w1_d = nc.dram_tensor("w1", [n_layers, P, DT, FF], BF, kind="ExternalInput").ap()
    w2_d = nc.dram_tensor("w2", [n_layers, P, FT, D], BF, kind="ExternalInput").ap()
    vecs_d = nc.dram_tensor("vecs", [6, n_layers, P, DT], F32, kind="ExternalInput").ap()
    b1_d = nc.dram_tensor("b1v", [n_layers, P, FT], F32, kind="ExternalInput").ap()
    bv_d = nc.dram_tensor("bvt", [n_layers, D], F32, kind="ExternalInput").ap()
    out_d = nc.dram_tensor("out", [P, DT, t_own], F32, kind="ExternalOutput").ap()

    import contextlib
    stack = contextlib.ExitStack()

    state = stack.enter_context(tc.tile_pool(name="state", bufs=1))
    wsmall = stack.enter_context(tc.tile_pool(name="wsmall", bufs=2))
    w1pool = stack.enter_context(tc.tile_pool(name="w1pool", bufs=1))
    w2pool = stack.enter_context(tc.tile_pool(name="w2pool", bufs=1))
    epool = stack.enter_context(tc.tile_pool(name="epool", bufs=4))
    hpool = stack.enter_context(tc.tile_pool(name="hpool", bufs=FT + 2))
    dpool = stack.enter_context(tc.tile_pool(name="dpool", bufs=2))
    bvpool = stack.enter_context(tc.tile_pool(name="bvpool", bufs=2))
    rpool = stack.enter_context(tc.tile_pool(name="rpool", bufs=3))
    scpool = stack.enter_context(tc.tile_pool(name="scpool", bufs=2, space="PSUM"))
    avpool = stack.enter_context(tc.tile_pool(name="avpool", bufs=2, space="PSUM"))
    mmpool = stack.enter_context(tc.tile_pool(name="mmpool", bufs=2, space="PSUM"))
    dram = stack.enter_context(tc.tile_pool(name="dram", bufs=2, space="DRAM"))

    # ---- persistent state ----------------------------------------------
    x_sb = state.tile([P, DT, t_own], F32)       # residual stream (f32)
    xq = state.tile([P, DT, t_own], F8)          # post-BN2 cast (q rhs / exchange)
    xbf_b = state.tile([P, DT, t_own], BF)       # post-BN1 cast (ffn rhs)
    kvx = state.tile([P, DT, s_kv], F8)          # kv-source activations (full seq)
    kT = state.tile([P, DT, s_kv], F8)           # K, feature-major
    q_sb = state.tile([P, DT, t_own], F8)        # Q, feature-major
    vplus = state.tile([P, KTP, 2, H, P], F8)    # V token-major, 128 cols/head
    attnT = state.tile([P, HP, t_own], BF)       # attention out, [dh+64*(h%2), hp, tok]

    vecs_sb = state.tile([P, 6, n_layers, DT], F32)
    nc.sync.dma_start(vecs_sb[:], vecs_d.rearrange("v l p f -> p v l f"))
    b1_sb = state.tile([P, n_layers, FT], F32)
    nc.sync.dma_start(b1_sb[:], b1_d.rearrange("l p f -> p l f"))

    BQ, BK, S1, BE1, S2, BE2 = range(6)

    # static parts of vplus: even head h: V at cols 0..63, ones at col 64
    # (denominator lands on psum partition 64); odd head: ones at col 0
    # (denominator on partition 0), V at cols 64..127.
    nc.vector.memset(vplus[:], 0.0)
    for h in range(H):
        col = 64 if h % 2 == 0 else 0
        nc.vector.memset(vplus[:, :, :, h, col : col + 1], 1.0)

    def layer(l):
        # ---- weights for this layer ---------------------------------
        wq_sb = wsmall.tile([P, DT, D], F8, tag="wq")
        nc.sync.dma_start(wq_sb[:], wq_d[l])
        wk_sb = wsmall.tile([P, DT, D], F8, tag="wk")
        nc.sync.dma_start(wk_sb[:], wk_d[l])
        wv_sb = wsmall.tile([P, DT, D], F8, tag="wv")
        nc.sync.dma_start(wv_sb[:], wv_d[l])
        wo_sb = wsmall.tile([P, HP, D], BF, tag="wo")
        nc.sync.dma_start(wo_sb[:], wo_d[l])
        w1_sb = w1pool.tile([P, DT, FF], BF, tag="w1")
        nc.sync.dma_start(w1_sb[:], w1_d[l])
        w2_sb = w2pool.tile([P, FT, D], BF, tag="w2")
        nc.sync.dma_start(w2_sb[:], w2_d[l])
        bvrow = bvpool.tile([1, D], F32, tag="bvrow")
        nc.sync.dma_start(bvrow[:], bv_d[l][None, :])
        bvb = bvpool.tile([P, D], F32, tag="bvb")
        nc.gpsimd.partition_broadcast(bvb[:], bvrow[:])

        # ---- kv-source for this layer -------------------------------
        if l == 0:
            nc.sync.dma_start(kvx[:], xkv0_d)

        # ---- projection piece emitters (all fp8 DoubleRow) ----------
        def kproj_piece(dt, st):
            ps = mmpool.tile([P, 512], F32, tag="mm", name=f"kp{l}_{dt}_{st}")
            for kp in range(DT // 2):
                nc.tensor.matmul(
                    ps[:],
                    wk_sb[:, 2 * kp : 2 * kp + 2, dt * P : (dt + 1) * P],
                    kvx[:, 2 * kp : 2 * kp + 2, st * 512 : (st + 1) * 512],
                    start=(kp == 0),
                    stop=(kp == DT // 2 - 1),
                    perf_mode=DR,
                )
            nc.vector.tensor_scalar(
                kT[:, dt, st * 512 : (st + 1) * 512],
                ps[:],
                vecs_sb[:, BK, l, dt : dt + 1],
                None,
                OP.add,
            )

        def vproj_piece(tt):
            ps = mmpool.tile([P, 512], F32, tag="mm", name=f"vp{l}_{tt}")
            for kp in range(DT // 2):
                nc.tensor.matmul(
                    ps[:],
                    kvx[:, 2 * kp : 2 * kp + 2, tt * P : (tt + 1) * P],
                    wv_sb[:, 2 * kp : 2 * kp + 2, :],
                    start=(kp == 0),
                    stop=(kp == DT // 2 - 1),
                    perf_mode=DR,
                )
            # even heads -> cols 0..63, odd heads -> cols 64..127 of their
            # 128-col slot; two strided passes (psum stays [tok, h*64+dh]).
            vdst = vplus[:, tt // 2, tt % 2].rearrange(
                "p (hp par) c -> p hp par c", par=2
            )
            psrc = ps[:].rearrange("p (hp par e) -> p hp par e", hp=HP, par=2)
            bsrc = bvb[:].rearrange("p (hp par e) -> p hp par e", hp=HP, par=2)
            for par in range(2):
                nc.vector.tensor_tensor(
                    vdst[:, :, par, par * 64 : par * 64 + 64],
                    psrc[:, :, par, :],
                    bsrc[:, :, par, :],
                    OP.add,
                )

        def qproj_piece(dt, st):
            ps = mmpool.tile([P, 512], F32, tag="mm", name=f"qp{l}_{dt}_{st}")
            for kp in range(DT // 2):
                nc.tensor.matmul(
                    ps[:],
                    wq_sb[:, 2 * kp : 2 * kp + 2, dt * P : (dt + 1) * P],
                    xq[:, 2 * kp : 2 * kp + 2, st * 512 : (st + 1) * 512],
                    start=(kp == 0),
                    stop=(kp == DT // 2 - 1),
                    perf_mode=DR,
                )
            nc.vector.tensor_scalar(
                q_sb[:, dt, st * 512 : (st + 1) * 512],
                ps[:],
                vecs_sb[:, BQ, l, dt : dt + 1],
                None,
                OP.add,
            )

        def oproj_piece(qt, dt):
            qsl = slice(qt * 512, (qt + 1) * 512)
            ps = mmpool.tile([P, 512], F32, tag="mm", name=f"op{l}_{qt}_{dt}")
            for hp in range(HP):
                nc.tensor.matmul(
                    ps[:],
                    wo_sb[:, hp, dt * P : (dt + 1) * P],
                    attnT[:, hp, qsl],
                    start=(hp == 0),
                    stop=(hp == HP - 1),
                )
            nc.vector.tensor_tensor(
                x_sb[:, dt, qsl], x_sb[:, dt, qsl], ps[:], OP.add
            )
            nc.vector.tensor_scalar(
                x_sb[:, dt, qsl],
                x_sb[:, dt, qsl],
                vecs_sb[:, S1, l, dt : dt + 1],
                vecs_sb[:, BE1, l, dt : dt + 1],
                OP.mult,
                OP.add,
            )
            nc.vector.tensor_copy(xbf_b[:, dt, qsl], x_sb[:, dt, qsl])

        hsbs = {}

        def ht_piece(qt, ft):
            qsl = slice(qt * 512, (qt + 1) * 512)
            hps = mmpool.tile([P, 512], F32, tag="mm", name=f"hp{l}_{qt}_{ft}")
            for kd in range(DT):
                nc.tensor.matmul(
                    hps[:],
                    w1_sb[:, kd, ft * P : (ft + 1) * P],
                    xbf_b[:, kd, qsl],
                    start=(kd == 0),
                    stop=(kd == DT - 1),
                )
            hsb = hpool.tile([P, 512], BF, tag="h", name=f"h{l}_{qt}_{ft}")
            nc.vector.tensor_scalar(
                hsb[:], hps[:], b1_sb[:, l, ft : ft + 1], 0.0, OP.add, OP.max
            )
            hsbs[(qt, ft)] = hsb

        fpss = {}

        def ft_piece_a(qt, dt):
            fps = mmpool.tile([P, 512], F32, tag="mm", name=f"ft{l}_{qt}_{dt}")
            for ft in range(FT // 2):
                nc.tensor.matmul(
                    fps[:],
                    w2_sb[:, ft, dt * P : (dt + 1) * P],
                    hsbs[(qt, ft)][:],
                    start=(ft == 0),
                    stop=False,
                )
            fpss[(qt, dt)] = fps

        def ft_piece_b(qt, dt):
            qsl = slice(qt * 512, (qt + 1) * 512)
            fps = fpss.pop((qt, dt))
            for ft in range(FT // 2, FT):
                nc.tensor.matmul(
                    fps[:],
                    w2_sb[:, ft, dt * P : (dt + 1) * P],
                    hsbs[(qt, ft)][:],
                    start=False,
                    stop=(ft == FT - 1),
                )
            nc.vector.tensor_tensor(
                x_sb[:, dt, qsl], x_sb[:, dt, qsl], fps[:], OP.add
            )
            nc.vector.tensor_scalar(
                x_sb[:, dt, qsl],
                x_sb[:, dt, qsl],
                vecs_sb[:, S2, l, dt : dt + 1],
                vecs_sb[:, BE2, l, dt : dt + 1],
                OP.mult,
                OP.add,
            )
            nc.vector.tensor_copy(xq[:, dt, qsl], x_sb[:, dt, qsl])

        def ft_piece(qt, dt):
            ft_piece_a(qt, dt)
            ft_piece_b(qt, dt)

        # ---- next-layer activation exchange (issued early) ----------
        def ag_piece(qt):
            if not use_ag or l == n_layers - 1:
                return
            qsl = slice(qt * 512, (qt + 1) * 512)
            bounce_in = dram.tile([P, DT, 512], F8, tag="agin", name=f"agin{l}_{qt}")
            bounce_out = dram.tile(
                [2, P, DT, 512], F8, tag="agout", name=f"agout{l}_{qt}"
            )
            nc.sync.dma_start(bounce_in[:], xq[:, :, qsl])
            nc.gpsimd.collective_compute(
                "AllGather",
                mybir.AluOpType.bypass,
                replica_groups=[[0, 1], [2, 3], [4, 5], [6, 7]],
                ins=[bounce_in[:].opt()],
                outs=[bounce_out[:].opt()],
            )
            for s in range(2):
                nc.sync.dma_start(
                    kvx[:, :, s * t_own + qt * 512 : s * t_own + (qt + 1) * 512],
                    bounce_out[s],
                )

        # ---- attention chunk for one (qt, hp), woven with filler ----
        # kt-tile PAIRS (ktp) are processed in exchange-arrival order.
        # Per ktp: the even- and odd-head score MMs are emitted
        # INTERLEAVED so adjacent MMs target different PE row groups and
        # run concurrently; each head's [P,1024] score tile (2 banks,
        # double-buffered) feeds one exp -> fp8 e tile -> one DoubleRow
        # AV MM.  Filler is drained between the MM groups so the PE has
        # queued work while the exps run.
        def attn_chunk(qt, hp, filler, per_kt, deadlines=False):
            qsl = slice(qt * 512, (qt + 1) * 512)
            av0 = avpool.tile([P, 512], F32, tag="av", name=f"av0_{l}_{qt}_{hp}")
            av1 = avpool.tile([P, 512], F32, tag="av", name=f"av1_{l}_{qt}_{hp}")
            state = {"budget": 0.0}

            def drain(frac):
                state["budget"] += frac
                while state["budget"] >= 1.0 and filler:
                    filler.pop(0)[1]()
                    state["budget"] -= 1.0

            for pos, ktp in enumerate(KTP_ORDER):
                if deadlines:
                    key = hp * KT + 2 * pos
                    while filler and filler[0][0] is not None and filler[0][0] <= key + 2:
                        filler.pop(0)[1]()
                        state["budget"] -= 1.0
                sc0 = scpool.tile([P, 1024], F32, tag="sc", name=f"sc0_{l}_{qt}_{hp}_{ktp}")
                sc1 = scpool.tile([P, 1024], F32, tag="sc", name=f"sc1_{l}_{qt}_{hp}_{ktp}")
                for j in range(2):
                    kt = 2 * ktp + j
                    nc.tensor.matmul(
                        sc0[:, j * 512 : (j + 1) * 512],
                        kT[0:DH, hp, kt * P : (kt + 1) * P],
                        q_sb[0:DH, hp, qsl],
                        start=True,
                        stop=True,
                    )
                    nc.tensor.matmul(
                        sc1[:, j * 512 : (j + 1) * 512],
                        kT[DH:P, hp, kt * P : (kt + 1) * P],
                        q_sb[DH:P, hp, qsl],
                        start=True,
                        stop=True,
                    )
                drain(per_kt / 2.0)
                e0 = epool.tile([P, 2, 512], F8, tag="e", name=f"e0_{l}_{qt}_{hp}_{ktp}")
                nc.scalar.activation(
                    e0[:].rearrange("p a b -> p (a b)"), sc0[:], AF.Exp,
                    scale=ATT_SCALE / (WS * WS),
                )
                e1 = epool.tile([P, 2, 512], F8, tag="e", name=f"e1_{l}_{qt}_{hp}_{ktp}")
                nc.scalar.activation(
                    e1[:].rearrange("p a b -> p (a b)"), sc1[:], AF.Exp,
                    scale=ATT_SCALE / (WS * WS),
                )
                nc.tensor.matmul(
                    av0[:],
                    vplus[:, ktp, :, 2 * hp, :],
                    e0[:],
                    start=(pos == 0),
                    stop=(pos == KTP - 1),
                    perf_mode=DR,
                )
                nc.tensor.matmul(
                    av1[:],
                    vplus[:, ktp, :, 2 * hp + 1, :],
                    e1[:],
                    start=(pos == 0),
                    stop=(pos == KTP - 1),
                    perf_mode=DR,
                )
                drain(per_kt / 2.0)
            # evacuate the accumulators to SBUF promptly so the next chunk's
            # AV matmuls get the psum banks back without waiting for the
            # whole normalize chain.
            avsb = dpool.tile([P, 2, 512], F32, tag="avsb", name=f"avs{l}_{qt}_{hp}")
            nc.vector.tensor_copy(avsb[:, 0, :], av0[:])
            nc.vector.tensor_copy(avsb[:, 1, :], av1[:])
            # denominators: even head at avsb[64,0], odd at avsb[0,1];
            # broadcast them raw, then invert full-width
            # (reciprocal_approx_fast only works on full-128-partition tiles).
            den0 = dpool.tile([1, 2, 512], F32, tag="den0", name=f"d0{l}_{qt}_{hp}")
            nc.sync.dma_start(den0[:, 0:1, :], avsb[64:65, 0:1, :])
            nc.sync.dma_start(den0[:, 1:2, :], avsb[0:1, 1:2, :])
            rbr = rpool.tile([P, 2, 512], F32, tag="rbr", name=f"rbr{l}_{qt}_{hp}")
            nc.gpsimd.partition_broadcast(rbr[:], den0[:])
            rb = rpool.tile([P, 2, 512], F32, tag="rb", name=f"rb{l}_{qt}_{hp}")
            nc.vector.reciprocal_approx_fast(rb[:], rbr[:])
            nc.vector.tensor_tensor(
                attnT[0:64, hp, qsl], avsb[0:64, 0, :], rb[0:64, 0, :], OP.mult
            )
            nc.vector.tensor_tensor(
                attnT[64:P, hp, qsl], avsb[64:P, 1, :], rb[64:P, 1, :], OP.mult
            )

        from functools import partial

        # lead-in: K/Q for head-pair 0, V for the first token tiles
        kproj_piece(0, 0)
        qproj_piece(0, 0)
        vproj_piece(0)
        vproj_piece(1)

        # filler for attention(qt0): remaining K/Q/V projections, each
        # tagged with the (hp*KTP + ktp) step of attention(qt0) that first
        # needs it (None = not needed until attention(qt1)).
        POS = {ktp: i for i, ktp in enumerate(KTP_ORDER)}
        fill0 = []
        for tt in range(2, KT):
            fill0.append((2 * POS[tt // 2], partial(vproj_piece, tt)))
        for dt in range(DT):
            for st in range(SKT):
                if dt == 0 and st == 0:
                    continue
                fill0.append(
                    (dt * KT + 2 * POS[2 * st], partial(kproj_piece, dt, st))
                )
            if dt > 0:
                fill0.append((dt * KT, partial(qproj_piece, dt, 0)))
        if QT > 1:
            for dt in range(DT):
                fill0.append((None, partial(qproj_piece, dt, 1)))
        fill0.sort(key=lambda t: t[0] if t[0] is not None else 10 ** 9)

        for hp in range(HP):
            per_kt0 = max(len(fill0) / ((HP - hp) * KTP), 0.01)
            attn_chunk(0, hp, fill0, per_kt0, deadlines=True)
        for _, f in fill0:
            f()
        fill0.clear()

        # attention(qt1) woven with o-proj + FFN of qt0; the next-layer
        # AllGather for qt0 fires as soon as ft_piece(0, *) are all done.
        if QT > 1:
            fill1 = [(None, partial(oproj_piece, 0, dt)) for dt in range(DT)]
            for ft in range(FT):
                fill1.append((None, partial(ht_piece, 0, ft)))
            for dt in range(DT):
                fill1.append((None, partial(ft_piece_a, 0, dt)))
                fill1.append((None, partial(ft_piece_b, 0, dt)))
            fill1.append((None, partial(ag_piece, 0)))
            for hp in range(HP):
                per_kt1 = max(len(fill1) / ((HP - hp) * KTP), 0.01)
                attn_chunk(1, hp, fill1, per_kt1)
            for _, f in fill1:
                f()
            fill1.clear()

        # tail: o-proj + FFN of the last qt, then its exchange
        last = QT - 1
        for dt in range(DT):
            oproj_piece(last, dt)
        for ft in range(FT):
            ht_piece(last, ft)
        for dt in range(DT):
            ft_piece(last, dt)
        ag_piece(last)

    # warm up the collective channels with two full-size AllGathers on the
    # same bounce-buffer slots the real exchanges will use, so the first
    # real exchange doesn't pay the ~60us cold-start.
    if use_ag and n_layers > 1:
        wu_sb = dpool.tile([P, DT, 512], F8, tag="wusb", name="wusb")
        nc.vector.memset(wu_sb[:], 0.0)
        for r in range(2):
            wu_in = dram.tile([P, DT, 512], F8, tag="agin", name=f"wuin{r}")
            wu_out = dram.tile([2, P, DT, 512], F8, tag="agout", name=f"wuout{r}")
            nc.sync.dma_start(wu_in[:], wu_sb[:])
            nc.gpsimd.collective_compute(
                "AllGather",
                mybir.AluOpType.bypass,
                replica_groups=[[0, 1], [2, 3], [4, 5], [6, 7]],
                ins=[wu_in[:].opt()],
                outs=[wu_out[:].opt()],
            )

    # initial load + cast
    nc.sync.dma_start(x_sb[:], x0_d)
    nc.vector.tensor_copy(xq[:], x_sb[:])

    for l in range(n_layers):
        layer(l)

    nc.sync.dma_start(out_d, x_sb[:])
    stack.close()


def _host_inputs(sequence, wq, bq, wk, bk, wv, bv, wo, bo, w1, b1, w2, b2,
                 g1, be1, g2, be2, *, n_layers=L, t_own=S // 2, s_kv=S,
                 use_ag=True, n_cores=N_CORES):
    """Build the shared + per-core input maps."""
    s1 = (g1 * BN_INV).astype(np.float32)
    be1p = (bo * s1 + be1).astype(np.float32)
    s2 = (g2 * BN_INV).astype(np.float32)
    be2p = (b2 * s2 + be2).astype(np.float32)

    vecs = np.stack([
        _vec_tiles(bq * WS), _vec_tiles(bk * WS),
        _vec_tiles(s1), _vec_tiles(be1p),
        _vec_tiles(s2), _vec_tiles(be2p),
    ]).astype(np.float32)                        # [6, L, 128, DT]

    shared = {
        "wq": np.stack([_w_tiles(wq[l] * WS) for l in range(n_layers)]).astype(FP8),
        "wk": np.stack([_w_tiles(wk[l] * WS) for l in range(n_layers)]).astype(FP8),
        "wv": np.stack([_w_tiles(wv[l] * WS) for l in range(n_layers)]).astype(FP8),
        "wo": np.stack(
            [_wo_tiles(wo[l] / WS) for l in range(n_layers)]
        ).astype(BF16),
        "w1": np.stack([_w_tiles(w1[l]) for l in range(n_layers)]).astype(BF16),
        "w2": np.stack([_w_tiles(w2[l]) for l in range(n_layers)]).astype(BF16),
        "vecs": vecs,
        "b1v": _vec_tiles(b1).astype(np.float32),
        "bvt": (bv * WS).astype(np.float32),
    }

    in_maps = []
    for i in range(n_cores):
        if use_ag:
            b, half = i // 2, i % 2
            tok = slice(half * t_own, (half + 1) * t_own)
        else:
            b, tok = i % sequence.shape[0], slice(0, t_own)
        m = dict(shared)
        m["x0"] = _fmajor(sequence[b][tok].astype(np.float32), t_own)
        m["xkv0"] = _fmajor(sequence[b][:s_kv], s_kv).astype(FP8)
        in_maps.append(m)
    return in_maps


def _assemble(results, *, t_own=S // 2, use_ag=True):
    out = np.zeros((B, S, D), np.float32)
    for i, r in enumerate(results):
        xo = r["out"]                        # [128, DT, t_own]
        xd = xo.transpose(1, 0, 2).reshape(DT * P, t_own).T   # [t_own, D]
        if use_ag:
            b, half = i // 2, i % 2
            out[b, half * t_own : (half + 1) * t_own] = xd
        else:
            if i < B:
                out[i, :t_own] = xd
    return out


def _build(n_layers=L, t_own=S // 2, s_kv=S, use_ag=True, n_cores=N_CORES):
    from concourse import bacc
    import concourse.tile as tile

    nc = bacc.Bacc(
        "TRN2",
        target_bir_lowering=False,
        debug=False,
        enable_asserts=False,
        num_devices=n_cores,
    )
    with tile.TileContext(nc) as tc:
        build_encoder(nc, tc, n_layers=n_layers, t_own=t_own, s_kv=s_kv,
                      use_ag=use_ag)
    nc.compile()
    return nc


def kernel(**inputs) -> np.ndarray:
    from concourse.bass_utils import run_bass_kernel_spmd

    use_ag = True
    t_own = S // 2
    nc = _build(use_ag=use_ag, t_own=t_own)
    in_maps = _host_inputs(**{k: np.asarray(v) for k, v in inputs.items()},
                           use_ag=use_ag, t_own=t_own)
    res = run_bass_kernel_spmd(nc, in_maps, core_ids=list(range(N_CORES)))
    return _assemble(res.results, t_own=t_own, use_ag=use_ag)


# revision 29
# speedup vs baseline: 1.2329x; 1.2329x over previous
"""Trainium2 Bass kernel for a 4-layer transformer encoder.

Model (hardcoded from the problem spec):
  L=4 layers, B=4, S=2048, D=512, H=8 heads (DH=64), FF=2048,
  inference BatchNorm with moving stats (0,1): bn(x) = x/sqrt(1+eps)*g + be.

Sharding: 8 cores. Cores (2b, 2b+1) handle batch item b; each computes
1024 of the item's 2048 tokens.  The residual stream is kept
feature-major [D, T] in SBUF (f32).  After each layer the pair
exchanges its updated activations (fp8) via a 2-rank AllGather through
DRAM bounce buffers so both cores have the full 2048-token sequence
for the next layer's K/V; the AllGather for (layer+1, qt) is issued as
soon as this layer's FFN for qt completes, hiding its latency under
the remaining attention / FFN work.

Precision strategy: the residual stream and all BN math stay f32; the
FFN and o-projection matmuls are bf16; Q/K/V projections, attention
scores and att@V run in fp8e4 with DoubleRow perf mode (two 128-deep
k-tiles per PE pass) for 2x matmul throughput.  fp8 errors inside
attention wash out in the softmax average and land on the (small)
sublayer outputs, not the residual stream.

Attention: scores are computed transposed [ktok, qtok] (contraction
DH=64; the two heads of a pair run concurrently on the top/bottom
halves of the PE array via row tiling), exp on ScalarE over [128,1024]
two-PSUM-bank tiles (scale folded in; logits are O(1) so no max
subtraction) emitting fp8, and att@V uses DoubleRow with a
ones-augmented, 128-column-per-head V so the softmax denominator
falls out of the same matmul: even heads occupy V columns 0..63 with
ones at column 64, odd heads have ones at column 63 and V at 64..127,
so the normalized outputs land on disjoint partition halves of attnT
and the o-projection contracts a full K=128 head pair per pass.
"""

import math

import numpy as np
import ml_dtypes

# ---- problem constants --------------------------------------------------
L, B, S, D, H = 4, 4, 2048, 512, 8
DH = D // H            # 64
FF = 4 * D             # 2048
BN_EPS = 1e-3
P = 128
DT = D // P            # 4 feature tiles
FT = FF // P           # 16 ffn tiles
HP = H // 2            # 4 head pairs
ATT_SCALE = 1.0 / math.sqrt(DH)
BN_INV = 1.0 / math.sqrt(1.0 + BN_EPS)
# fp8 weight pre-scale: raw qkv weights (std ~0.02) sit in e4m3's subnormal
# range; scale them up before casting, compensate via the exp() scale (q/k)
# and a folded 1/WS in wo (v path).
WS = 32.0
import os
EXP2BANK = os.environ.get("EXP2BANK", "1") == "1"

N_CORES = 8

BF16 = ml_dtypes.bfloat16
FP8 = ml_dtypes.float8_e4m3


def _fmajor(a, t):
    """[T, D_any] -> feature-major [128, D_any//128, T] tile layout."""
    d = a.shape[1]
    return np.ascontiguousarray(a.T.reshape(d // P, P, t).transpose(1, 0, 2))


def _w_tiles(w):
    """[K, N] weight -> [128, K//128, N] (partition = K within tile)."""
    k, n = w.shape
    return np.ascontiguousarray(w.reshape(k // P, P, n).transpose(1, 0, 2))


def _wo_tiles(w):
    """[D, D] attention out-proj -> [128, HP, D] with partition =
    dh + 64*(h%2), tile index = h//2 (matches attnT's layout)."""
    t = np.zeros((P, HP, D), w.dtype)
    for h in range(H):
        t[64 * (h % 2) : 64 * (h % 2) + 64, h // 2, :] = w[h * DH : (h + 1) * DH, :]
    return t


def _vec_tiles(v):
    """[L, D_any] -> [L, 128, D_any//128] per-partition layout."""
    l, d = v.shape
    return np.ascontiguousarray(v.reshape(l, d // P, P).transpose(0, 2, 1))


def build_encoder(nc, tc, *, n_layers, t_own, s_kv, use_ag):
    """Emit the encoder onto TileContext tc.  t_own = tokens this core
    computes; s_kv = tokens attended over (= 2*t_own when use_ag)."""
    import concourse.bass as bass
    import concourse.mybir as mybir
    from concourse.bass import ds, ts

    F32 = mybir.dt.float32
    BF = mybir.dt.bfloat16
    F8 = mybir.dt.float8e4
    AF = mybir.ActivationFunctionType
    OP = mybir.AluOpType
    DR = mybir.MatmulPerfMode.DoubleRow

    KT = s_kv // P          # 16 ktok 128-tiles
    KTP = KT // 2           # 8 ktok 256-pairs (DoubleRow step)
    SKT = s_kv // 512       # 4 kv projection stream tiles
    QT = t_own // 512       # 2 qtok stream tiles
    # process k-token stream chunks in exchange-arrival order: the qt0
    # AllGather is prefetched a layer earlier and lands first, the qt1 one
    # only finishes ~20us into the layer.
    ST_ORDER = [0, 2, 1, 3] if SKT == 4 else list(range(SKT))
    ARR = {st: i for i, st in enumerate(ST_ORDER)}
    KTP_ORDER = [k for st in ST_ORDER for k in (2 * st, 2 * st + 1)]
    # step index at which k-stream chunk st is first consumed
    ST_STEP = {st: 2 * i for i, st in enumerate(ST_ORDER)}

    # ---- dram I/O -------------------------------------------------------
    x0_d = nc.dram_tensor("x0", [P, DT, t_own], F32, kind="ExternalInput").ap()
    xkv0_d = nc.dram_tensor("xkv0", [P, DT, s_kv], F8, kind="ExternalInput").ap()
    wq_d = nc.dram_tensor("wq", [n_layers, P, DT, D], F8, kind="ExternalInput").ap()
    wk_d = nc.dram_tensor("wk", [n_layers, P, DT, D], F8, kind="ExternalInput").ap()
    wv_d = nc.dram_tensor("wv", [n_layers, P, DT, D], F8, kind="ExternalInput").ap()
    wo_d = nc.dram_tensor("wo", [n_layers, P, HP, D], BF, kind="ExternalInput").ap()
    w1_d = nc.dram_tensor("w1", [n_layers, P, DT, FF], BF, kind="ExternalInput").ap()
    w2_d = nc.dram_tensor("w2", [n_layers, P, FT, D], BF, kind="ExternalInput").ap()
    vecs_d = nc.dram_tensor("vecs", [6, n_layers, P, DT], F32, kind="ExternalInput").ap()
    b1_d = nc.dram_tensor("b1v", [n_layers, P, FT], F32, kind="ExternalInput").ap()
    bv_d = nc.dram_tensor("bvt", [n_layers, D], F32, kind="ExternalInput").ap()
    out_d = nc.dram_tensor("out", [P, DT, t_own], F32, kind="ExternalOutput").ap()

    import contextlib
    stack = contextlib.ExitStack()

    state = stack.enter_context(tc.tile_pool(name="state", bufs=1))
    wsmall = stack.enter_context(tc.tile_pool(name="wsmall", bufs=2))
    w1pool = stack.enter_context(tc.tile_pool(name="w1pool", bufs=1))
    w2pool = stack.enter_context(tc.tile_pool(name="w2pool", bufs=1))
    epool = stack.enter_context(tc.tile_pool(name="epool", bufs=4))
    hpool = stack.enter_context(tc.tile_pool(name="hpool", bufs=FT + 2))
    dpool = stack.enter_context(tc.tile_pool(name="dpool", bufs=2))
    avsbpool = stack.enter_context(tc.tile_pool(name="avsbpool", bufs=HP + 2))
    bvpool = stack.enter_context(tc.tile_pool(name="bvpool", bufs=2))
    rpool = stack.enter_context(tc.tile_pool(name="rpool", bufs=2))
    scpool = stack.enter_context(tc.tile_pool(name="scpool", bufs=2, space="PSUM"))
    avpool = stack.enter_context(tc.tile_pool(name="avpool", bufs=2, space="PSUM"))
    mmpool = stack.enter_context(tc.tile_pool(name="mmpool", bufs=2, space="PSUM"))
    dram = stack.enter_context(tc.tile_pool(name="dram", bufs=2, space="DRAM"))

    # ---- persistent state ----------------------------------------------
    x_sb = state.tile([P, DT, t_own], F32)       # residual stream (f32)
    xq = state.tile([P, DT, t_own], F8)          # post-BN2 cast (q rhs / exchange)
    xbf_b = state.tile([P, DT, t_own], BF)       # post-BN1 cast (ffn rhs)
    kvx = state.tile([P, DT, s_kv], F8)          # kv-source activations (full seq)
    kT = state.tile([P, DT, s_kv], F8)           # K, feature-major
    q_sb = state.tile([P, DT, t_own], F8)        # Q, feature-major
    vplus = state.tile([P, KTP, 2, H, P], F8)    # V token-major, 128 cols/head
    attnT = state.tile([P, HP, t_own], BF)       # attention out, [dh+64*(h%2), hp, tok]

    vecs_sb = state.tile([P, 6, n_layers, DT], F32)
    nc.sync.dma_start(vecs_sb[:], vecs_d.rearrange("v l p f -> p v l f"))
    b1_sb = state.tile([P, n_layers, FT], F32)
    nc.sync.dma_start(b1_sb[:], b1_d.rearrange("l p f -> p l f"))

    BQ, BK, S1, BE1, S2, BE2 = range(6)

    # static parts of vplus: even head h: V at cols 0..63, ones at col 64
    # (denominator lands on psum partition 64); odd head: ones at col 0
    # (denominator on partition 0), V at cols 64..127.
    nc.vector.memset(vplus[:], 0.0)
    for h in range(H):
        col = 64 if h % 2 == 0 else 0
        nc.vector.memset(vplus[:, :, :, h, col : col + 1], 1.0)

    def layer(l):
        # ---- weights for this layer ---------------------------------
        wq_sb = wsmall.tile([P, DT, D], F8, tag="wq")
        nc.sync.dma_start(wq_sb[:], wq_d[l])
        wk_sb = wsmall.tile([P, DT, D], F8, tag="wk")
        nc.sync.dma_start(wk_sb[:], wk_d[l])
        wv_sb = wsmall.tile([P, DT, D], F8, tag="wv")
        nc.sync.dma_start(wv_sb[:], wv_d[l])
        wo_sb = wsmall.tile([P, HP, D], BF, tag="wo")
        nc.sync.dma_start(wo_sb[:], wo_d[l])
        w1_sb = w1pool.tile([P, DT, FF], BF, tag="w1")
        nc.sync.dma_start(w1_sb[:], w1_d[l])
        w2_sb = w2pool.tile([P, FT, D], BF, tag="w2")
        nc.sync.dma_start(w2_sb[:], w2_d[l])
        bvrow = bvpool.tile([1, D], F32, tag="bvrow")
        nc.sync.dma_start(bvrow[:], bv_d[l][None, :])
        bvb = bvpool.tile([P, D], F32, tag="bvb")
        nc.gpsimd.partition_broadcast(bvb[:], bvrow[:])

        # ---- kv-source for this layer -------------------------------
        if l == 0:
            nc.sync.dma_start(kvx[:], xkv0_d)

        # ---- projection piece emitters (all fp8 DoubleRow) ----------
        def kproj_piece(dt, st):
            ps = mmpool.tile([P, 512], F32, tag="mm", name=f"kp{l}_{dt}_{st}")
            for kp in range(DT // 2):
                nc.tensor.matmul(
                    ps[:],
                    wk_sb[:, 2 * kp : 2 * kp + 2, dt * P : (dt + 1) * P],
                    kvx[:, 2 * kp : 2 * kp + 2, st * 512 : (st + 1) * 512],
                    start=(kp == 0),
                    stop=(kp == DT // 2 - 1),
                    perf_mode=DR,
                )
            nc.vector.tensor_scalar(
                kT[:, dt, st * 512 : (st + 1) * 512],
                ps[:],
                vecs_sb[:, BK, l, dt : dt + 1],
                None,
                OP.add,
            )

        def vproj_piece(tt):
            ps = mmpool.tile([P, 512], F32, tag="mm", name=f"vp{l}_{tt}")
            for kp in range(DT // 2):
                nc.tensor.matmul(
                    ps[:],
                    kvx[:, 2 * kp : 2 * kp + 2, tt * P : (tt + 1) * P],
                    wv_sb[:, 2 * kp : 2 * kp + 2, :],
                    start=(kp == 0),
                    stop=(kp == DT // 2 - 1),
                    perf_mode=DR,
                )
            # even heads -> cols 0..63, odd heads -> cols 64..127 of their
            # 128-col slot; two strided passes (psum stays [tok, h*64+dh]).
            vdst = vplus[:, tt // 2, tt % 2].rearrange(
                "p (hp par) c -> p hp par c", par=2
            )
            psrc = ps[:].rearrange("p (hp par e) -> p hp par e", hp=HP, par=2)
            bsrc = bvb[:].rearrange("p (hp par e) -> p hp par e", hp=HP, par=2)
            for par in range(2):
                nc.vector.tensor_tensor(
                    vdst[:, :, par, par * 64 : par * 64 + 64],
                    psrc[:, :, par, :],
                    bsrc[:, :, par, :],
                    OP.add,
                )

        def qproj_piece(dt, st):
            ps = mmpool.tile([P, 512], F32, tag="mm", name=f"qp{l}_{dt}_{st}")
            for kp in range(DT // 2):
                nc.tensor.matmul(
                    ps[:],
                    wq_sb[:, 2 * kp : 2 * kp + 2, dt * P : (dt + 1) * P],
                    xq[:, 2 * kp : 2 * kp + 2, st * 512 : (st + 1) * 512],
                    start=(kp == 0),
                    stop=(kp == DT // 2 - 1),
                    perf_mode=DR,
                )
            nc.vector.tensor_scalar(
                q_sb[:, dt, st * 512 : (st + 1) * 512],
                ps[:],
                vecs_sb[:, BQ, l, dt : dt + 1],
                None,
                OP.add,
            )

        def oproj_piece(qt, dt):
            qsl = slice(qt * 512, (qt + 1) * 512)
            ps = mmpool.tile([P, 512], F32, tag="mm", name=f"op{l}_{qt}_{dt}")
            for hp in range(HP):
                nc.tensor.matmul(
                    ps[:],
                    wo_sb[:, hp, dt * P : (dt + 1) * P],
                    attnT[:, hp, qsl],
                    start=(hp == 0),
                    stop=(hp == HP - 1),
                )
            nc.vector.tensor_tensor(
                x_sb[:, dt, qsl], x_sb[:, dt, qsl], ps[:], OP.add
            )
            nc.vector.tensor_scalar(
                x_sb[:, dt, qsl],
                x_sb[:, dt, qsl],
                vecs_sb[:, S1, l, dt : dt + 1],
                vecs_sb[:, BE1, l, dt : dt + 1],
                OP.mult,
                OP.add,
            )
            nc.vector.tensor_copy(xbf_b[:, dt, qsl], x_sb[:, dt, qsl])

        hsbs = {}

        def ht_piece(qt, ft):
            qsl = slice(qt * 512, (qt + 1) * 512)
            hps = mmpool.tile([P, 512], F32, tag="mm", name=f"hp{l}_{qt}_{ft}")
            for kd in range(DT):
                nc.tensor.matmul(
                    hps[:],
                    w1_sb[:, kd, ft * P : (ft + 1) * P],
                    xbf_b[:, kd, qsl],
                    start=(kd == 0),
                    stop=(kd == DT - 1),
                )
            hsb = hpool.tile([P, 512], BF, tag="h", name=f"h{l}_{qt}_{ft}")
            nc.vector.tensor_scalar(
                hsb[:], hps[:], b1_sb[:, l, ft : ft + 1], 0.0, OP.add, OP.max
            )
            hsbs[(qt, ft)] = hsb

        fpss = {}

        def ft_piece_a(qt, dt):
            fps = mmpool.tile([P, 512], F32, tag="mm", name=f"ft{l}_{qt}_{dt}")
            for ft in range(FT // 2):
                nc.tensor.matmul(
                    fps[:],
                    w2_sb[:, ft, dt * P : (dt + 1) * P],
                    hsbs[(qt, ft)][:],
                    start=(ft == 0),
                    stop=False,
                )
            fpss[(qt, dt)] = fps

        def ft_piece_b(qt, dt):
            qsl = slice(qt * 512, (qt + 1) * 512)
            fps = fpss.pop((qt, dt))
            for ft in range(FT // 2, FT):
                nc.tensor.matmul(
                    fps[:],
                    w2_sb[:, ft, dt * P : (dt + 1) * P],
                    hsbs[(qt, ft)][:],
                    start=False,
                    stop=(ft == FT - 1),
                )
            nc.vector.tensor_tensor(
                x_sb[:, dt, qsl], x_sb[:, dt, qsl], fps[:], OP.add
            )
            nc.vector.tensor_scalar(
                x_sb[:, dt, qsl],
                x_sb[:, dt, qsl],
                vecs_sb[:, S2, l, dt : dt + 1],
                vecs_sb[:, BE2, l, dt : dt + 1],
                OP.mult,
                OP.add,
            )
            nc.vector.tensor_copy(xq[:, dt, qsl], x_sb[:, dt, qsl])

        def ft_piece(qt, dt):
            ft_piece_a(qt, dt)
            ft_piece_b(qt, dt)

        # ---- next-layer activation exchange (issued early) ----------
        def ag_piece(qt):
            if not use_ag or l == n_layers - 1:
                return
            qsl = slice(qt * 512, (qt + 1) * 512)
            bounce_in = dram.tile([P, DT, 512], F8, tag="agin", name=f"agin{l}_{qt}")
            bounce_out = dram.tile(
                [2, P, DT, 512], F8, tag="agout", name=f"agout{l}_{qt}"
            )
            nc.sync.dma_start(bounce_in[:], xq[:, :, qsl])
            nc.gpsimd.collective_compute(
                "AllGather",
                mybir.AluOpType.bypass,
                replica_groups=[[0, 1], [2, 3], [4, 5], [6, 7]],
                ins=[bounce_in[:].opt()],
                outs=[bounce_out[:].opt()],
            )
            for s in range(2):
                nc.sync.dma_start(
                    kvx[:, :, s * t_own + qt * 512 : s * t_own + (qt + 1) * 512],
                    bounce_out[s],
                )

        # ---- attention part for one (qt, hp), woven with filler ----
        # kt-tile PAIRS (ktp) are processed in exchange-arrival order.
        # Per ktp: the even- and odd-head score MMs are emitted
        # INTERLEAVED so adjacent MMs target different PE row groups and
        # run concurrently; each head's [P,1024] score tile (2 banks,
        # double-buffered) feeds one exp -> fp8 e tile -> one DoubleRow
        # AV MM.  Filler is drained between the MM groups so the PE has
        # queued work while the exps run.  A chunk may be split into
        # [lo,hi) position parts: partial accumulators are evacuated to
        # SBUF between parts (freeing the psum banks) and summed at the
        # end, which lets all chunks' exchange-independent halves run
        # before any exchange-dependent position is needed.
        def attn_part(qt, hp, lo, hi, filler, per_kt, deadlines=False, avsb=None):
            qsl = slice(qt * 512, (qt + 1) * 512)
            av0 = avpool.tile([P, 512], F32, tag="av", name=f"av0_{l}_{qt}_{hp}_{lo}")
            av1 = avpool.tile([P, 512], F32, tag="av", name=f"av1_{l}_{qt}_{hp}_{lo}")
            state = {"budget": 0.0}

            def drain(frac):
                state["budget"] += frac
                while state["budget"] >= 1.0 and filler:
                    filler.pop(0)[1]()
                    state["budget"] -= 1.0

            for pos in range(lo, hi):
                ktp = KTP_ORDER[pos]
                if deadlines:
                    key = hp * 2 * (hi - lo) + 2 * (pos - lo)
                    while filler and filler[0][0] is not None and filler[0][0] <= key + 2:
                        filler.pop(0)[1]()
                        state["budget"] -= 1.0
                sc0 = scpool.tile([P, 1024], F32, tag="sc", name=f"sc0_{l}_{qt}_{hp}_{ktp}")
                sc1 = scpool.tile([P, 1024], F32, tag="sc", name=f"sc1_{l}_{qt}_{hp}_{ktp}")
                for j in range(2):
                    kt = 2 * ktp + j
                    nc.tensor.matmul(
                        sc0[:, j * 512 : (j + 1) * 512],
                        kT[0:DH, hp, kt * P : (kt + 1) * P],
                        q_sb[0:DH, hp, qsl],
                        start=True,
                        stop=True,
                    )
                    nc.tensor.matmul(
                        sc1[:, j * 512 : (j + 1) * 512],
                        kT[DH:P, hp, kt * P : (kt + 1) * P],
                        q_sb[DH:P, hp, qsl],
                        start=True,
                        stop=True,
                    )
                drain(per_kt / 2.0)
                e0 = epool.tile([P, 2, 512], F8, tag="e", name=f"e0_{l}_{qt}_{hp}_{ktp}")
                nc.scalar.activation(
                    e0[:].rearrange("p a b -> p (a b)"), sc0[:], AF.Exp,
                    scale=ATT_SCALE / (WS * WS),
                )
                e1 = epool.tile([P, 2, 512], F8, tag="e", name=f"e1_{l}_{qt}_{hp}_{ktp}")
                nc.scalar.activation(
                    e1[:].rearrange("p a b -> p (a b)"), sc1[:], AF.Exp,
                    scale=ATT_SCALE / (WS * WS),
                )
                nc.tensor.matmul(
                    av0[:],
                    vplus[:, ktp, :, 2 * hp, :],
                    e0[:],
                    start=(pos == lo),
                    stop=(pos == hi - 1),
                    perf_mode=DR,
                )
                nc.tensor.matmul(
                    av1[:],
                    vplus[:, ktp, :, 2 * hp + 1, :],
                    e1[:],
                    start=(pos == lo),
                    stop=(pos == hi - 1),
                    perf_mode=DR,
                )
                drain(per_kt / 2.0)
            # evacuate the accumulators to SBUF promptly so the next part's
            # AV matmuls get the psum banks back without waiting for the
            # whole normalize chain.  First part copies, later parts add.
            if avsb is None:
                avsb = avsbpool.tile(
                    [P, 2, 512], F32, tag="avsb", name=f"avs{l}_{qt}_{hp}_{lo}"
                )
                nc.vector.tensor_copy(avsb[:, 0, :], av0[:])
                nc.vector.tensor_copy(avsb[:, 1, :], av1[:])
            else:
                nc.vector.tensor_tensor(avsb[:, 0, :], avsb[:, 0, :], av0[:], OP.add)
                nc.vector.tensor_tensor(avsb[:, 1, :], avsb[:, 1, :], av1[:], OP.add)
            if hi < KTP:
                return avsb
            # denominators: even head at avsb[64,0], odd at avsb[0,1];
            # broadcast them raw, then invert full-width
            # (reciprocal_approx_fast only works on full-128-partition tiles).
            den0 = dpool.tile([1, 2, 512], F32, tag="den0", name=f"d0{l}_{qt}_{hp}")
            nc.sync.dma_start(den0[:, 0:1, :], avsb[64:65, 0:1, :])
            nc.sync.dma_start(den0[:, 1:2, :], avsb[0:1, 1:2, :])
            rbr = rpool.tile([P, 2, 512], F32, tag="rbr", name=f"rbr{l}_{qt}_{hp}")
            nc.gpsimd.partition_broadcast(rbr[:], den0[:])
            rb = rpool.tile([P, 2, 512], F32, tag="rb", name=f"rb{l}_{qt}_{hp}")
            nc.vector.reciprocal_approx_fast(rb[:], rbr[:])
            nc.vector.tensor_tensor(
                attnT[0:64, hp, qsl], avsb[0:64, 0, :], rb[0:64, 0, :], OP.mult
            )
            nc.vector.tensor_tensor(
                attnT[64:P, hp, qsl], avsb[64:P, 1, :], rb[64:P, 1, :], OP.mult
            )

        from functools import partial

        POS = {ktp: i for i, ktp in enumerate(KTP_ORDER)}

        def _drain_all(fl):
            for _, f in fl:
                f()
            fl.clear()

        if SKT == 4:
            # -- phase-split qt0 attention: every chunk's exchange-
            # independent half (positions 0..3 = st0/st2) runs first, so
            # the qt1 exchange (sent at the end of the previous layer by
            # BOTH pair cores) has ~40us of slack before any position
            # needs it, riding out partner-core skew.
            kproj_piece(0, 0)
            qproj_piece(0, 0)
            vproj_piece(0)
            vproj_piece(1)

            fill0a = []
            for tt in range(2, KT):
                if POS[tt // 2] < 4:
                    fill0a.append((2 * POS[tt // 2], partial(vproj_piece, tt)))
            for dt in range(DT):
                for st in (0, 2):
                    if dt == 0 and st == 0:
                        continue
                    fill0a.append(
                        (dt * 8 + 2 * POS[2 * st], partial(kproj_piece, dt, st))
                    )
                if dt > 0:
                    fill0a.append((dt * 8, partial(qproj_piece, dt, 0)))
            fill0a.sort(key=lambda t: t[0])

            avsbs = {}
            for hp in range(HP):
                per_kt = max(len(fill0a) / ((HP - hp) * 4), 0.01)
                avsbs[hp] = attn_part(0, hp, 0, 4, fill0a, per_kt, deadlines=True)
            _drain_all(fill0a)

            fill0b = []
            for tt in range(2, KT):
                if POS[tt // 2] >= 4:
                    fill0b.append(
                        (2 * (POS[tt // 2] - 4), partial(vproj_piece, tt))
                    )
            for dt in range(DT):
                for st in (1, 3):
                    fill0b.append(
                        (dt * 8 + 2 * (POS[2 * st] - 4), partial(kproj_piece, dt, st))
                    )
            if QT > 1:
                for dt in range(DT):
                    fill0b.append((None, partial(qproj_piece, dt, 1)))
            fill0b.sort(key=lambda t: t[0] if t[0] is not None else 10 ** 9)

            for hp in range(HP):
                per_kt = max(len(fill0b) / ((HP - hp) * 4), 0.01)
                attn_part(0, hp, 4, KTP, fill0b, per_kt, deadlines=True,
                          avsb=avsbs.pop(hp))
            _drain_all(fill0b)
        else:
            kproj_piece(0, 0)
            qproj_piece(0, 0)
            vproj_piece(0)
            vproj_piece(1)
            fill0 = []
            for tt in range(2, KT):
                fill0.append((2 * POS[tt // 2], partial(vproj_piece, tt)))
            for dt in range(DT):
                for st in range(SKT):
                    if dt == 0 and st == 0:
                        continue
                    fill0.append(
                        (dt * 2 * KTP + 2 * POS[2 * st], partial(kproj_piece, dt, st))
                    )
                if dt > 0:
                    fill0.append((dt * 2 * KTP, partial(qproj_piece, dt, 0)))
            if QT > 1:
                for dt in range(DT):
                    fill0.append((None, partial(qproj_piece, dt, 1)))
            fill0.sort(key=lambda t: t[0] if t[0] is not None else 10 ** 9)
            for hp in range(HP):
                per_kt0 = max(len(fill0) / ((HP - hp) * KTP), 0.01)
                attn_part(0, hp, 0, KTP, fill0, per_kt0, deadlines=True)
            _drain_all(fill0)

        # attention(qt1) woven with o-proj + FFN of qt0; the next-layer
        # AllGather for qt0 fires as soon as ft_piece(0, *) are all done.
        if QT > 1:
            fill1 = [(None, partial(oproj_piece, 0, dt)) for dt in range(DT)]
            for ft in range(FT):
                fill1.append((None, partial(ht_piece, 0, ft)))
            for dt in range(DT):
                fill1.append((None, partial(ft_piece_a, 0, dt)))
                fill1.append((None, partial(ft_piece_b, 0, dt)))
            fill1.append((None, partial(ag_piece, 0)))
            for hp in range(HP):
                per_kt1 = max(len(fill1) / ((HP - hp) * KTP), 0.01)
                attn_part(1, hp, 0, KTP, fill1, per_kt1)
            _drain_all(fill1)

        # tail: o-proj + FFN of the last qt, then its exchange
        last = QT - 1
        for dt in range(DT):
            oproj_piece(last, dt)
        for ft in range(FT):
            ht_piece(last, ft)
        for dt in range(DT):
            ft_piece(last, dt)
        ag_piece(last)

    # warm up the collective channels with two full-size AllGathers on the
    # same bounce-buffer slots the real exchanges will use, so the first
    # real exchange doesn't pay the ~60us cold-start.
    if use_ag and n_layers > 1:
        wu_sb = dpool.tile([P, DT, 512], F8, tag="wusb", name="wusb")
        nc.vector.memset(wu_sb[:], 0.0)
        for r in range(2):
            wu_in = dram.tile([P, DT, 512], F8, tag="agin", name=f"wuin{r}")
            wu_out = dram.tile([2, P, DT, 512], F8, tag="agout", name=f"wuout{r}")
            nc.sync.dma_start(wu_in[:], wu_sb[:])
            nc.gpsimd.collective_compute(
                "AllGather",
                mybir.AluOpType.bypass,
                replica_groups=[[0, 1], [2, 3], [4, 5], [6, 7]],
                ins=[wu_in[:].opt()],
                outs=[wu_out[:].opt()],
            )

    # initial load + cast
    nc.sync.dma_start(x_sb[:], x0_d)
    nc.vector.tensor_copy(xq[:], x_sb[:])

    for l in range(n_layers):
        layer(l)

    nc.sync.dma_start(out_d, x_sb[:])
    stack.close()


def _host_inputs(sequence, wq, bq, wk, bk, wv, bv, wo, bo, w1, b1, w2, b2,
                 g1, be1, g2, be2, *, n_layers=L, t_own=S // 2, s_kv=S,
                 use_ag=True, n_cores=N_CORES):
    """Build the shared + per-core input maps."""
    s1 = (g1 * BN_INV).astype(np.float32)
    be1p = (bo * s1 + be1).astype(np.float32)
    s2 = (g2 * BN_INV).astype(np.float32)
    be2p = (b2 * s2 + be2).astype(np.float32)

    vecs = np.stack([
        _vec_tiles(bq * WS), _vec_tiles(bk * WS),
        _vec_tiles(s1), _vec_tiles(be1p),
        _vec_tiles(s2), _vec_tiles(be2p),
    ]).astype(np.float32)                        # [6, L, 128, DT]

    shared = {
        "wq": np.stack([_w_tiles(wq[l] * WS) for l in range(n_layers)]).astype(FP8),
        "wk": np.stack([_w_tiles(wk[l] * WS) for l in range(n_layers)]).astype(FP8),
        "wv": np.stack([_w_tiles(wv[l] * WS) for l in range(n_layers)]).astype(FP8),
        "wo": np.stack(
            [_wo_tiles(wo[l] / WS) for l in range(n_layers)]
        ).astype(BF16),
        "w1": np.stack([_w_tiles(w1[l]) for l in range(n_layers)]).astype(BF16),
        "w2": np.stack([_w_tiles(w2[l]) for l in range(n_layers)]).astype(BF16),
        "vecs": vecs,
        "b1v": _vec_tiles(b1).astype(np.float32),
        "bvt": (bv * WS).astype(np.float32),
    }

    in_maps = []
    for i in range(n_cores):
        if use_ag:
            b, half = i // 2, i % 2
            tok = slice(half * t_own, (half + 1) * t_own)
        else:
            b, tok = i % sequence.shape[0], slice(0, t_own)
        m = dict(shared)
        m["x0"] = _fmajor(sequence[b][tok].astype(np.float32), t_own)
        m["xkv0"] = _fmajor(sequence[b][:s_kv], s_kv).astype(FP8)
        in_maps.append(m)
    return in_maps


def _assemble(results, *, t_own=S // 2, use_ag=True):
    out = np.zeros((B, S, D), np.float32)
    for i, r in enumerate(results):
        xo = r["out"]                        # [128, DT, t_own]
        xd = xo.transpose(1, 0, 2).reshape(DT * P, t_own).T   # [t_own, D]
        if use_ag:
            b, half = i // 2, i % 2
            out[b, half * t_own : (half + 1) * t_own] = xd
        else:
            if i < B:
                out[i, :t_own] = xd
    return out


def _build(n_layers=L, t_own=S // 2, s_kv=S, use_ag=True, n_cores=N_CORES):
    from concourse import bacc
    import concourse.tile as tile

    nc = bacc.Bacc(
        "TRN2",
        target_bir_lowering=False,
        debug=False,
        enable_asserts=False,
        num_devices=n_cores,
    )
    with tile.TileContext(nc) as tc:
        build_encoder(nc, tc, n_layers=n_layers, t_own=t_own, s_kv=s_kv,
                      use_ag=use_ag)
    nc.compile()
    return nc


def kernel(**inputs) -> np.ndarray:
    from concourse.bass_utils import run_bass_kernel_spmd

    use_ag = True
    t_own = S // 2
    nc = _build(use_ag=use_ag, t_own=t_own)
    in_maps = _host_inputs(**{k: np.asarray(v) for k, v in inputs.items()},
                           use_ag=use_ag, t_own=t_own)
    res = run_bass_kernel_spmd(nc, in_maps, core_ids=list(range(N_CORES)))
    return _assemble(res.results, t_own=t_own, use_ag=use_ag)


# revision 30
# speedup vs baseline: 1.2402x; 1.0059x over previous
"""Trainium2 Bass kernel for a 4-layer transformer encoder.

Model (hardcoded from the problem spec):
  L=4 layers, B=4, S=2048, D=512, H=8 heads (DH=64), FF=2048,
  inference BatchNorm with moving stats (0,1): bn(x) = x/sqrt(1+eps)*g + be.

Sharding: 8 cores. Cores (2b, 2b+1) handle batch item b; each computes
1024 of the item's 2048 tokens.  The residual stream is kept
feature-major [D, T] in SBUF (f32).  After each layer the pair
exchanges its updated activations (fp8) via a 2-rank AllGather through
DRAM bounce buffers so both cores have the full 2048-token sequence
for the next layer's K/V; the AllGather for (layer+1, qt) is issued as
soon as this layer's FFN for qt completes, hiding its latency under
the remaining attention / FFN work.

Precision strategy: the residual stream and all BN math stay f32; the
FFN and o-projection matmuls are bf16; Q/K/V projections, attention
scores and att@V run in fp8e4 with DoubleRow perf mode (two 128-deep
k-tiles per PE pass) for 2x matmul throughput.  fp8 errors inside
attention wash out in the softmax average and land on the (small)
sublayer outputs, not the residual stream.

Attention: scores are computed transposed [ktok, qtok] (contraction
DH=64; the two heads of a pair run concurrently on the top/bottom
halves of the PE array via row tiling), exp on ScalarE over [128,1024]
two-PSUM-bank tiles (scale folded in; logits are O(1) so no max
subtraction) emitting fp8, and att@V uses DoubleRow with a
ones-augmented, 128-column-per-head V so the softmax denominator
falls out of the same matmul: even heads occupy V columns 0..63 with
ones at column 64, odd heads have ones at column 63 and V at 64..127,
so the normalized outputs land on disjoint partition halves of attnT
and the o-projection contracts a full K=128 head pair per pass.
"""

import math

import numpy as np
import ml_dtypes

# ---- problem constants --------------------------------------------------
L, B, S, D, H = 4, 4, 2048, 512, 8
DH = D // H            # 64
FF = 4 * D             # 2048
BN_EPS = 1e-3
P = 128
DT = D // P            # 4 feature tiles
FT = FF // P           # 16 ffn tiles
HP = H // 2            # 4 head pairs
ATT_SCALE = 1.0 / math.sqrt(DH)
BN_INV = 1.0 / math.sqrt(1.0 + BN_EPS)
# fp8 weight pre-scale: raw qkv weights (std ~0.02) sit in e4m3's subnormal
# range; scale them up before casting, compensate via the exp() scale (q/k)
# and a folded 1/WS in wo (v path).
WS = 32.0
import os
EXP2BANK = os.environ.get("EXP2BANK", "1") == "1"

N_CORES = 8

BF16 = ml_dtypes.bfloat16
FP8 = ml_dtypes.float8_e4m3


def _fmajor(a, t):
    """[T, D_any] -> feature-major [128, D_any//128, T] tile layout."""
    d = a.shape[1]
    return np.ascontiguousarray(a.T.reshape(d // P, P, t).transpose(1, 0, 2))


def _w_tiles(w):
    """[K, N] weight -> [128, K//128, N] (partition = K within tile)."""
    k, n = w.shape
    return np.ascontiguousarray(w.reshape(k // P, P, n).transpose(1, 0, 2))


def _wo_tiles(w):
    """[D, D] attention out-proj -> [128, HP, D] with partition =
    dh + 64*(h%2), tile index = h//2 (matches attnT's layout)."""
    t = np.zeros((P, HP, D), w.dtype)
    for h in range(H):
        t[64 * (h % 2) : 64 * (h % 2) + 64, h // 2, :] = w[h * DH : (h + 1) * DH, :]
    return t


def _vec_tiles(v):
    """[L, D_any] -> [L, 128, D_any//128] per-partition layout."""
    l, d = v.shape
    return np.ascontiguousarray(v.reshape(l, d // P, P).transpose(0, 2, 1))


def build_encoder(nc, tc, *, n_layers, t_own, s_kv, use_ag):
    """Emit the encoder onto TileContext tc.  t_own = tokens this core
    computes; s_kv = tokens attended over (= 2*t_own when use_ag)."""
    import concourse.bass as bass
    import concourse.mybir as mybir
    from concourse.bass import ds, ts

    F32 = mybir.dt.float32
    BF = mybir.dt.bfloat16
    F8 = mybir.dt.float8e4
    AF = mybir.ActivationFunctionType
    OP = mybir.AluOpType
    DR = mybir.MatmulPerfMode.DoubleRow

    KT = s_kv // P          # 16 ktok 128-tiles
    KTP = KT // 2           # 8 ktok 256-pairs (DoubleRow step)
    SKT = s_kv // 512       # 4 kv projection stream tiles
    QT = t_own // 512       # 2 qtok stream tiles
    # process k-token stream chunks in exchange-arrival order: the qt0
    # AllGather is prefetched a layer earlier and lands first, the qt1 one
    # only finishes ~20us into the layer.
    ST_ORDER = [0, 2, 1, 3] if SKT == 4 else list(range(SKT))
    ARR = {st: i for i, st in enumerate(ST_ORDER)}
    KTP_ORDER = [k for st in ST_ORDER for k in (2 * st, 2 * st + 1)]
    # step index at which k-stream chunk st is first consumed
    ST_STEP = {st: 2 * i for i, st in enumerate(ST_ORDER)}

    # ---- dram I/O -------------------------------------------------------
    x0_d = nc.dram_tensor("x0", [P, DT, t_own], F32, kind="ExternalInput").ap()
    xkv0_d = nc.dram_tensor("xkv0", [P, DT, s_kv], F8, kind="ExternalInput").ap()
    wq_d = nc.dram_tensor("wq", [n_layers, P, DT, D], F8, kind="ExternalInput").ap()
    wk_d = nc.dram_tensor("wk", [n_layers, P, DT, D], F8, kind="ExternalInput").ap()
    wv_d = nc.dram_tensor("wv", [n_layers, P, DT, D], F8, kind="ExternalInput").ap()
    wo_d = nc.dram_tensor("wo", [n_layers, P, HP, D], BF, kind="ExternalInput").ap()
    w1_d = nc.dram_tensor("w1", [n_layers, P, DT, FF], BF, kind="ExternalInput").ap()
    w2_d = nc.dram_tensor("w2", [n_layers, P, FT, D], BF, kind="ExternalInput").ap()
    vecs_d = nc.dram_tensor("vecs", [6, n_layers, P, DT], F32, kind="ExternalInput").ap()
    b1_d = nc.dram_tensor("b1v", [n_layers, P, FT], F32, kind="ExternalInput").ap()
    bv_d = nc.dram_tensor("bvt", [n_layers, D], F32, kind="ExternalInput").ap()
    out_d = nc.dram_tensor("out", [P, DT, t_own], F32, kind="ExternalOutput").ap()

    import contextlib
    stack = contextlib.ExitStack()

    state = stack.enter_context(tc.tile_pool(name="state", bufs=1))
    wsmall = stack.enter_context(tc.tile_pool(name="wsmall", bufs=2))
    w1pool = stack.enter_context(tc.tile_pool(name="w1pool", bufs=1))
    w2pool = stack.enter_context(tc.tile_pool(name="w2pool", bufs=1))
    epool = stack.enter_context(tc.tile_pool(name="epool", bufs=4))
    hpool = stack.enter_context(tc.tile_pool(name="hpool", bufs=FT + 2))
    dpool = stack.enter_context(tc.tile_pool(name="dpool", bufs=2))
    avsbpool = stack.enter_context(tc.tile_pool(name="avsbpool", bufs=HP + 2))
    bvpool = stack.enter_context(tc.tile_pool(name="bvpool", bufs=2))
    rpool = stack.enter_context(tc.tile_pool(name="rpool", bufs=2))
    scpool = stack.enter_context(tc.tile_pool(name="scpool", bufs=2, space="PSUM"))
    avpool = stack.enter_context(tc.tile_pool(name="avpool", bufs=2, space="PSUM"))
    mmpool = stack.enter_context(tc.tile_pool(name="mmpool", bufs=2, space="PSUM"))
    dram = stack.enter_context(tc.tile_pool(name="dram", bufs=2, space="DRAM"))

    # ---- persistent state ----------------------------------------------
    x_sb = state.tile([P, DT, t_own], F32)       # residual stream (f32)
    xq = state.tile([P, DT, t_own], F8)          # post-BN2 cast (q rhs / exchange)
    xbf_b = state.tile([P, DT, t_own], BF)       # post-BN1 cast (ffn rhs)
    kvx = state.tile([P, DT, s_kv], F8)          # kv-source activations (full seq)
    kT = state.tile([P, DT, s_kv], F8)           # K, feature-major
    q_sb = state.tile([P, DT, t_own], F8)        # Q, feature-major
    vplus = state.tile([P, KTP, 2, H, P], F8)    # V token-major, 128 cols/head
    attnT = state.tile([P, HP, t_own], BF)       # attention out, [dh+64*(h%2), hp, tok]

    vecs_sb = state.tile([P, 6, n_layers, DT], F32)
    nc.sync.dma_start(vecs_sb[:], vecs_d.rearrange("v l p f -> p v l f"))
    b1_sb = state.tile([P, n_layers, FT], F32)
    nc.sync.dma_start(b1_sb[:], b1_d.rearrange("l p f -> p l f"))

    BQ, BK, S1, BE1, S2, BE2 = range(6)

    # static parts of vplus: even head h: V at cols 0..63, ones at col 64
    # (denominator lands on psum partition 64); odd head: ones at col 0
    # (denominator on partition 0), V at cols 64..127.
    nc.vector.memset(vplus[:], 0.0)
    for h in range(H):
        col = 64 if h % 2 == 0 else 0
        nc.vector.memset(vplus[:, :, :, h, col : col + 1], 1.0)

    def layer(l):
        # ---- weights for this layer ---------------------------------
        wq_sb = wsmall.tile([P, DT, D], F8, tag="wq")
        nc.sync.dma_start(wq_sb[:], wq_d[l])
        wk_sb = wsmall.tile([P, DT, D], F8, tag="wk")
        nc.sync.dma_start(wk_sb[:], wk_d[l])
        wv_sb = wsmall.tile([P, DT, D], F8, tag="wv")
        nc.sync.dma_start(wv_sb[:], wv_d[l])
        wo_sb = wsmall.tile([P, HP, D], BF, tag="wo")
        nc.sync.dma_start(wo_sb[:], wo_d[l])
        w1_sb = w1pool.tile([P, DT, FF], BF, tag="w1")
        nc.sync.dma_start(w1_sb[:], w1_d[l])
        w2_sb = w2pool.tile([P, FT, D], BF, tag="w2")
        nc.sync.dma_start(w2_sb[:], w2_d[l])
        bvrow = bvpool.tile([1, D], F32, tag="bvrow")
        nc.sync.dma_start(bvrow[:], bv_d[l][None, :])
        bvb = bvpool.tile([P, D], F32, tag="bvb")
        nc.gpsimd.partition_broadcast(bvb[:], bvrow[:])

        # ---- kv-source for this layer -------------------------------
        if l == 0:
            nc.sync.dma_start(kvx[:], xkv0_d)

        # ---- projection piece emitters (all fp8 DoubleRow) ----------
        def kproj_piece(dt, st):
            ps = mmpool.tile([P, 512], F32, tag="mm", name=f"kp{l}_{dt}_{st}")
            for kp in range(DT // 2):
                nc.tensor.matmul(
                    ps[:],
                    wk_sb[:, 2 * kp : 2 * kp + 2, dt * P : (dt + 1) * P],
                    kvx[:, 2 * kp : 2 * kp + 2, st * 512 : (st + 1) * 512],
                    start=(kp == 0),
                    stop=(kp == DT // 2 - 1),
                    perf_mode=DR,
                )
            nc.vector.tensor_scalar(
                kT[:, dt, st * 512 : (st + 1) * 512],
                ps[:],
                vecs_sb[:, BK, l, dt : dt + 1],
                None,
                OP.add,
            )

        def vproj_piece(tt):
            ps = mmpool.tile([P, 512], F32, tag="mm", name=f"vp{l}_{tt}")
            for kp in range(DT // 2):
                nc.tensor.matmul(
                    ps[:],
                    kvx[:, 2 * kp : 2 * kp + 2, tt * P : (tt + 1) * P],
                    wv_sb[:, 2 * kp : 2 * kp + 2, :],
                    start=(kp == 0),
                    stop=(kp == DT // 2 - 1),
                    perf_mode=DR,
                )
            # even heads -> cols 0..63, odd heads -> cols 64..127 of their
            # 128-col slot; two strided passes (psum stays [tok, h*64+dh]).
            vdst = vplus[:, tt // 2, tt % 2].rearrange(
                "p (hp par) c -> p hp par c", par=2
            )
            psrc = ps[:].rearrange("p (hp par e) -> p hp par e", hp=HP, par=2)
            bsrc = bvb[:].rearrange("p (hp par e) -> p hp par e", hp=HP, par=2)
            for par in range(2):
                nc.vector.tensor_tensor(
                    vdst[:, :, par, par * 64 : par * 64 + 64],
                    psrc[:, :, par, :],
                    bsrc[:, :, par, :],
                    OP.add,
                )

        def qproj_piece(dt, st):
            ps = mmpool.tile([P, 512], F32, tag="mm", name=f"qp{l}_{dt}_{st}")
            for kp in range(DT // 2):
                nc.tensor.matmul(
                    ps[:],
                    wq_sb[:, 2 * kp : 2 * kp + 2, dt * P : (dt + 1) * P],
                    xq[:, 2 * kp : 2 * kp + 2, st * 512 : (st + 1) * 512],
                    start=(kp == 0),
                    stop=(kp == DT // 2 - 1),
                    perf_mode=DR,
                )
            nc.vector.tensor_scalar(
                q_sb[:, dt, st * 512 : (st + 1) * 512],
                ps[:],
                vecs_sb[:, BQ, l, dt : dt + 1],
                None,
                OP.add,
            )

        def oproj_piece(qt, dt):
            qsl = slice(qt * 512, (qt + 1) * 512)
            ps = mmpool.tile([P, 512], F32, tag="mm", name=f"op{l}_{qt}_{dt}")
            for hp in range(HP):
                nc.tensor.matmul(
                    ps[:],
                    wo_sb[:, hp, dt * P : (dt + 1) * P],
                    attnT[:, hp, qsl],
                    start=(hp == 0),
                    stop=(hp == HP - 1),
                )
            nc.vector.tensor_tensor(
                x_sb[:, dt, qsl], x_sb[:, dt, qsl], ps[:], OP.add
            )
            nc.vector.tensor_scalar(
                x_sb[:, dt, qsl],
                x_sb[:, dt, qsl],
                vecs_sb[:, S1, l, dt : dt + 1],
                vecs_sb[:, BE1, l, dt : dt + 1],
                OP.mult,
                OP.add,
            )
            nc.vector.tensor_copy(xbf_b[:, dt, qsl], x_sb[:, dt, qsl])

        hsbs = {}

        def ht_piece(qt, ft):
            qsl = slice(qt * 512, (qt + 1) * 512)
            hps = mmpool.tile([P, 512], F32, tag="mm", name=f"hp{l}_{qt}_{ft}")
            for kd in range(DT):
                nc.tensor.matmul(
                    hps[:],
                    w1_sb[:, kd, ft * P : (ft + 1) * P],
                    xbf_b[:, kd, qsl],
                    start=(kd == 0),
                    stop=(kd == DT - 1),
                )
            hsb = hpool.tile([P, 512], BF, tag="h", name=f"h{l}_{qt}_{ft}")
            nc.vector.tensor_scalar(
                hsb[:], hps[:], b1_sb[:, l, ft : ft + 1], 0.0, OP.add, OP.max
            )
            hsbs[(qt, ft)] = hsb

        fpss = {}

        def ft_piece_a(qt, dt):
            fps = mmpool.tile([P, 512], F32, tag="mm", name=f"ft{l}_{qt}_{dt}")
            for ft in range(FT // 2):
                nc.tensor.matmul(
                    fps[:],
                    w2_sb[:, ft, dt * P : (dt + 1) * P],
                    hsbs[(qt, ft)][:],
                    start=(ft == 0),
                    stop=False,
                )
            fpss[(qt, dt)] = fps

        def ft_piece_b(qt, dt):
            qsl = slice(qt * 512, (qt + 1) * 512)
            fps = fpss.pop((qt, dt))
            for ft in range(FT // 2, FT):
                nc.tensor.matmul(
                    fps[:],
                    w2_sb[:, ft, dt * P : (dt + 1) * P],
                    hsbs[(qt, ft)][:],
                    start=False,
                    stop=(ft == FT - 1),
                )
            nc.vector.tensor_tensor(
                x_sb[:, dt, qsl], x_sb[:, dt, qsl], fps[:], OP.add
            )
            nc.vector.tensor_scalar(
                x_sb[:, dt, qsl],
                x_sb[:, dt, qsl],
                vecs_sb[:, S2, l, dt : dt + 1],
                vecs_sb[:, BE2, l, dt : dt + 1],
                OP.mult,
                OP.add,
            )
            nc.vector.tensor_copy(xq[:, dt, qsl], x_sb[:, dt, qsl])

        def ft_piece(qt, dt):
            ft_piece_a(qt, dt)
            ft_piece_b(qt, dt)

        # ---- next-layer activation exchange (issued early) ----------
        def ag_piece(qt):
            if not use_ag or l == n_layers - 1:
                return
            qsl = slice(qt * 512, (qt + 1) * 512)
            bounce_in = dram.tile([P, DT, 512], F8, tag="agin", name=f"agin{l}_{qt}")
            bounce_out = dram.tile(
                [2, P, DT, 512], F8, tag="agout", name=f"agout{l}_{qt}"
            )
            nc.sync.dma_start(bounce_in[:], xq[:, :, qsl])
            nc.gpsimd.collective_compute(
                "AllGather",
                mybir.AluOpType.bypass,
                replica_groups=[[0, 1], [2, 3], [4, 5], [6, 7]],
                ins=[bounce_in[:].opt()],
                outs=[bounce_out[:].opt()],
            )
            for s in range(2):
                nc.sync.dma_start(
                    kvx[:, :, s * t_own + qt * 512 : s * t_own + (qt + 1) * 512],
                    bounce_out[s],
                )

        # ---- attention part for one (qt, hp), woven with filler ----
        # kt-tile PAIRS (ktp) are processed in exchange-arrival order.
        # Per ktp: the even- and odd-head score MMs are emitted
        # INTERLEAVED so adjacent MMs target different PE row groups and
        # run concurrently; each head's [P,1024] score tile (2 banks,
        # double-buffered) feeds one exp -> fp8 e tile -> one DoubleRow
        # AV MM.  Filler is drained between the MM groups so the PE has
        # queued work while the exps run.  A chunk may be split into
        # [lo,hi) position parts: partial accumulators are evacuated to
        # SBUF between parts (freeing the psum banks) and summed at the
        # end, which lets all chunks' exchange-independent halves run
        # before any exchange-dependent position is needed.
        def attn_part(qt, hp, lo, hi, filler, per_kt, deadlines=False, avsb=None):
            qsl = slice(qt * 512, (qt + 1) * 512)
            av0 = avpool.tile([P, 512], F32, tag="av", name=f"av0_{l}_{qt}_{hp}_{lo}")
            av1 = avpool.tile([P, 512], F32, tag="av", name=f"av1_{l}_{qt}_{hp}_{lo}")
            state = {"budget": 0.0}

            def drain(frac):
                state["budget"] += frac
                while state["budget"] >= 1.0 and filler:
                    filler.pop(0)[1]()
                    state["budget"] -= 1.0

            for pos in range(lo, hi):
                ktp = KTP_ORDER[pos]
                if deadlines:
                    key = hp * 2 * (hi - lo) + 2 * (pos - lo)
                    while filler and filler[0][0] is not None and filler[0][0] <= key + 2:
                        filler.pop(0)[1]()
                        state["budget"] -= 1.0
                sc0 = scpool.tile([P, 1024], F32, tag="sc", name=f"sc0_{l}_{qt}_{hp}_{ktp}")
                sc1 = scpool.tile([P, 1024], F32, tag="sc", name=f"sc1_{l}_{qt}_{hp}_{ktp}")
                for j in range(2):
                    kt = 2 * ktp + j
                    nc.tensor.matmul(
                        sc0[:, j * 512 : (j + 1) * 512],
                        kT[0:DH, hp, kt * P : (kt + 1) * P],
                        q_sb[0:DH, hp, qsl],
                        start=True,
                        stop=True,
                    )
                    nc.tensor.matmul(
                        sc1[:, j * 512 : (j + 1) * 512],
                        kT[DH:P, hp, kt * P : (kt + 1) * P],
                        q_sb[DH:P, hp, qsl],
                        start=True,
                        stop=True,
                    )
                drain(per_kt / 2.0)
                e0 = epool.tile([P, 2, 512], F8, tag="e", name=f"e0_{l}_{qt}_{hp}_{ktp}")
                nc.scalar.activation(
                    e0[:].rearrange("p a b -> p (a b)"), sc0[:], AF.Exp,
                    scale=ATT_SCALE / (WS * WS),
                )
                e1 = epool.tile([P, 2, 512], F8, tag="e", name=f"e1_{l}_{qt}_{hp}_{ktp}")
                nc.scalar.activation(
                    e1[:].rearrange("p a b -> p (a b)"), sc1[:], AF.Exp,
                    scale=ATT_SCALE / (WS * WS),
                )
                nc.tensor.matmul(
                    av0[:],
                    vplus[:, ktp, :, 2 * hp, :],
                    e0[:],
                    start=(pos == lo),
                    stop=(pos == hi - 1),
                    perf_mode=DR,
                )
                nc.tensor.matmul(
                    av1[:],
                    vplus[:, ktp, :, 2 * hp + 1, :],
                    e1[:],
                    start=(pos == lo),
                    stop=(pos == hi - 1),
                    perf_mode=DR,
                )
                drain(per_kt / 2.0)
            # evacuate the accumulators to SBUF promptly so the next part's
            # AV matmuls get the psum banks back without waiting for the
            # whole normalize chain.  First part copies, later parts add.
            if avsb is None:
                avsb = avsbpool.tile(
                    [P, 2, 512], F32, tag="avsb", name=f"avs{l}_{qt}_{hp}_{lo}"
                )
                nc.vector.tensor_copy(avsb[:, 0, :], av0[:])
                nc.vector.tensor_copy(avsb[:, 1, :], av1[:])
            else:
                nc.vector.tensor_tensor(avsb[:, 0, :], avsb[:, 0, :], av0[:], OP.add)
                nc.vector.tensor_tensor(avsb[:, 1, :], avsb[:, 1, :], av1[:], OP.add)
            if hi < KTP:
                return avsb
            # denominators: even head at avsb[64,0], odd at avsb[0,1];
            # broadcast them raw, then invert full-width
            # (reciprocal_approx_fast only works on full-128-partition tiles).
            den0 = dpool.tile([1, 2, 512], F32, tag="den0", name=f"d0{l}_{qt}_{hp}")
            nc.sync.dma_start(den0[:, 0:1, :], avsb[64:65, 0:1, :])
            nc.sync.dma_start(den0[:, 1:2, :], avsb[0:1, 1:2, :])
            rbr = rpool.tile([P, 2, 512], F32, tag="rbr", name=f"rbr{l}_{qt}_{hp}")
            nc.gpsimd.partition_broadcast(rbr[:], den0[:])
            rb = rpool.tile([P, 2, 512], F32, tag="rb", name=f"rb{l}_{qt}_{hp}")
            nc.vector.reciprocal_approx_fast(rb[:], rbr[:])
            nc.vector.tensor_tensor(
                attnT[0:64, hp, qsl], avsb[0:64, 0, :], rb[0:64, 0, :], OP.mult
            )
            nc.vector.tensor_tensor(
                attnT[64:P, hp, qsl], avsb[64:P, 1, :], rb[64:P, 1, :], OP.mult
            )

        from functools import partial

        POS = {ktp: i for i, ktp in enumerate(KTP_ORDER)}

        def _drain_all(fl):
            for _, f in fl:
                f()
            fl.clear()

        if SKT == 4:
            # -- phase-split qt0 attention: every chunk's exchange-
            # independent half (positions 0..3 = st0/st2) runs first, so
            # the qt1 exchange (sent at the end of the previous layer by
            # BOTH pair cores) has ~40us of slack before any position
            # needs it, riding out partner-core skew.
            kproj_piece(0, 0)
            qproj_piece(0, 0)
            vproj_piece(0)
            vproj_piece(1)

            fill0a = []
            for tt in range(2, KT):
                if POS[tt // 2] < 4:
                    fill0a.append((2 * POS[tt // 2], partial(vproj_piece, tt)))
            for dt in range(DT):
                for st in (0, 2):
                    if dt == 0 and st == 0:
                        continue
                    fill0a.append(
                        (dt * 8 + 2 * POS[2 * st], partial(kproj_piece, dt, st))
                    )
                if dt > 0:
                    fill0a.append((dt * 8, partial(qproj_piece, dt, 0)))
            fill0a.sort(key=lambda t: t[0])

            avsbs = {}
            for hp in range(HP):
                per_kt = max(len(fill0a) / ((HP - hp) * 4), 0.01)
                avsbs[hp] = attn_part(0, hp, 0, 4, fill0a, per_kt, deadlines=True)
            _drain_all(fill0a)

            fill0b = []
            for tt in range(2, KT):
                if POS[tt // 2] >= 4:
                    fill0b.append(
                        (2 * (POS[tt // 2] - 4), partial(vproj_piece, tt))
                    )
            for dt in range(DT):
                for st in (1, 3):
                    fill0b.append(
                        (dt * 8 + 2 * (POS[2 * st] - 4), partial(kproj_piece, dt, st))
                    )
            if QT > 1:
                for dt in range(DT):
                    fill0b.append((None, partial(qproj_piece, dt, 1)))
            fill0b.sort(key=lambda t: t[0] if t[0] is not None else 10 ** 9)

            for hp in range(HP):
                per_kt = max(0.8 * len(fill0b) / ((HP - hp) * 4), 0.01)
                attn_part(0, hp, 4, KTP, fill0b, per_kt, deadlines=True,
                          avsb=avsbs.pop(hp))
            _drain_all(fill0b)
        else:
            kproj_piece(0, 0)
            qproj_piece(0, 0)
            vproj_piece(0)
            vproj_piece(1)
            fill0 = []
            for tt in range(2, KT):
                fill0.append((2 * POS[tt // 2], partial(vproj_piece, tt)))
            for dt in range(DT):
                for st in range(SKT):
                    if dt == 0 and st == 0:
                        continue
                    fill0.append(
                        (dt * 2 * KTP + 2 * POS[2 * st], partial(kproj_piece, dt, st))
                    )
                if dt > 0:
                    fill0.append((dt * 2 * KTP, partial(qproj_piece, dt, 0)))
            if QT > 1:
                for dt in range(DT):
                    fill0.append((None, partial(qproj_piece, dt, 1)))
            fill0.sort(key=lambda t: t[0] if t[0] is not None else 10 ** 9)
            for hp in range(HP):
                per_kt0 = max(len(fill0) / ((HP - hp) * KTP), 0.01)
                attn_part(0, hp, 0, KTP, fill0, per_kt0, deadlines=True)
            _drain_all(fill0)

        # attention(qt1) woven with o-proj + FFN of qt0; the next-layer
        # AllGather for qt0 fires as soon as ft_piece(0, *) are all done.
        if QT > 1:
            fill1 = [(None, partial(oproj_piece, 0, dt)) for dt in range(DT)]
            for ft in range(FT):
                fill1.append((None, partial(ht_piece, 0, ft)))
            for dt in range(DT):
                fill1.append((None, partial(ft_piece_a, 0, dt)))
                fill1.append((None, partial(ft_piece_b, 0, dt)))
            fill1.append((None, partial(ag_piece, 0)))
            for hp in range(HP):
                per_kt1 = max(0.82 * len(fill1) / ((HP - hp) * KTP), 0.01)
                attn_part(1, hp, 0, KTP, fill1, per_kt1)
            _drain_all(fill1)

        # tail: o-proj + FFN of the last qt, then its exchange
        last = QT - 1
        for dt in range(DT):
            oproj_piece(last, dt)
        for ft in range(FT):
            ht_piece(last, ft)
        for dt in range(DT):
            ft_piece(last, dt)
        ag_piece(last)

    # warm up the collective channels with two full-size AllGathers on the
    # same bounce-buffer slots the real exchanges will use, so the first
    # real exchange doesn't pay the ~60us cold-start.
    if use_ag and n_layers > 1:
        wu_sb = dpool.tile([P, DT, 512], F8, tag="wusb", name="wusb")
        nc.vector.memset(wu_sb[:], 0.0)
        for r in range(2):
            wu_in = dram.tile([P, DT, 512], F8, tag="agin", name=f"wuin{r}")
            wu_out = dram.tile([2, P, DT, 512], F8, tag="agout", name=f"wuout{r}")
            nc.sync.dma_start(wu_in[:], wu_sb[:])
            nc.gpsimd.collective_compute(
                "AllGather",
                mybir.AluOpType.bypass,
                replica_groups=[[0, 1], [2, 3], [4, 5], [6, 7]],
                ins=[wu_in[:].opt()],
                outs=[wu_out[:].opt()],
            )

    # initial load + cast
    nc.sync.dma_start(x_sb[:], x0_d)
    nc.vector.tensor_copy(xq[:], x_sb[:])

    for l in range(n_layers):
        layer(l)

    nc.sync.dma_start(out_d, x_sb[:])
    stack.close()


def _host_inputs(sequence, wq, bq, wk, bk, wv, bv, wo, bo, w1, b1, w2, b2,
                 g1, be1, g2, be2, *, n_layers=L, t_own=S // 2, s_kv=S,
                 use_ag=True, n_cores=N_CORES):
    """Build the shared + per-core input maps."""
    s1 = (g1 * BN_INV).astype(np.float32)
    be1p = (bo * s1 + be1).astype(np.float32)
    s2 = (g2 * BN_INV).astype(np.float32)
    be2p = (b2 * s2 + be2).astype(np.float32)

    vecs = np.stack([
        _vec_tiles(bq * WS), _vec_tiles(bk * WS),
        _vec_tiles(s1), _vec_tiles(be1p),
        _vec_tiles(s2), _vec_tiles(be2p),
    ]).astype(np.float32)                        # [6, L, 128, DT]

    shared = {
        "wq": np.stack([_w_tiles(wq[l] * WS) for l in range(n_layers)]).astype(FP8),
        "wk": np.stack([_w_tiles(wk[l] * WS) for l in range(n_layers)]).astype(FP8),
        "wv": np.stack([_w_tiles(wv[l] * WS) for l in range(n_layers)]).astype(FP8),
        "wo": np.stack(
            [_wo_tiles(wo[l] / WS) for l in range(n_layers)]
        ).astype(BF16),
        "w1": np.stack([_w_tiles(w1[l]) for l in range(n_layers)]).astype(BF16),
        "w2": np.stack([_w_tiles(w2[l]) for l in range(n_layers)]).astype(BF16),
        "vecs": vecs,
        "b1v": _vec_tiles(b1).astype(np.float32),
        "bvt": (bv * WS).astype(np.float32),
    }

    in_maps = []
    for i in range(n_cores):
        if use_ag:
            b, half = i // 2, i % 2
            tok = slice(half * t_own, (half + 1) * t_own)
        else:
            b, tok = i % sequence.shape[0], slice(0, t_own)
        m = dict(shared)
        m["x0"] = _fmajor(sequence[b][tok].astype(np.float32), t_own)
        m["xkv0"] = _fmajor(sequence[b][:s_kv], s_kv).astype(FP8)
        in_maps.append(m)
    return in_maps


def _assemble(results, *, t_own=S // 2, use_ag=True):
    out = np.zeros((B, S, D), np.float32)
    for i, r in enumerate(results):
        xo = r["out"]                        # [128, DT, t_own]
        xd = xo.transpose(1, 0, 2).reshape(DT * P, t_own).T   # [t_own, D]
        if use_ag:
            b, half = i // 2, i % 2
            out[b, half * t_own : (half + 1) * t_own] = xd
        else:
            if i < B:
                out[i, :t_own] = xd
    return out


def _build(n_layers=L, t_own=S // 2, s_kv=S, use_ag=True, n_cores=N_CORES):
    from concourse import bacc
    import concourse.tile as tile

    nc = bacc.Bacc(
        "TRN2",
        target_bir_lowering=False,
        debug=False,
        enable_asserts=False,
        num_devices=n_cores,
    )
    with tile.TileContext(nc) as tc:
        build_encoder(nc, tc, n_layers=n_layers, t_own=t_own, s_kv=s_kv,
                      use_ag=use_ag)
    nc.compile()
    return nc


def kernel(**inputs) -> np.ndarray:
    from concourse.bass_utils import run_bass_kernel_spmd

    use_ag = True
    t_own = S // 2
    nc = _build(use_ag=use_ag, t_own=t_own)
    in_maps = _host_inputs(**{k: np.asarray(v) for k, v in inputs.items()},
                           use_ag=use_ag, t_own=t_own)
    res = run_bass_kernel_spmd(nc, in_maps, core_ids=list(range(N_CORES)))
    return _assemble(res.results, t_own=t_own, use_ag=use_ag)
